# revision 51
# baseline (speedup 1.0000x reference)
"""Chamfer-style point loss (nn_PointLoss) on 8 Trainium2 NeuronCores.

Math (reference): reflect points across plane n.x+d=0; half1 = reflected
points (valid where s=p.n+d < 0, mask m1), half2 = original points (mask
m2 = ~m1). D[i,j] = ||half1[i]-half2[j]||^2. Output scalar =
50*(sum_j min_i(D) m2_j / c2 + sum_i min_j(D) m1_i / c1).

v4 device formulation: F[i,j] = rr1[i] + rr2[j] + a_i.(-2 b_j) with
penalty P=2^14 added to masked-out rows/cols, computed as one K=16 bf16
hi/lo matmul per (128,512) tile and min-reductions (no negation trick).
Points are laid out partition-inner (pt = 128*j + p) so the K-major
operand images are built with PE transposes of the on-chip composites —
no DRAM round trips, no scatter DMAs. Column mins finish with 4 more PE
transposes + free-axis reductions (no partition_all_reduce). Each core
outputs its row-min partials (negated, f32), its masked per-partition
column sum, and the row mask; kernel() gathers the 8 outputs and does
the final 8-way min-combine + masked means in numpy. No collectives —
device time stays decoupled from inter-core launch skew.

Sharding: half2 (column) axis split 8 ways, 512 cols/core; every core
holds all rows.
"""

import os
import sys

import numpy as np

for _p in ("/opt/trn_rl_repo", "/root/.axon_site/_ro/trn_rl_repo"):
    if os.path.isdir(_p) and _p not in sys.path:
        sys.path.insert(0, _p)

import concourse.bacc as bacc
import concourse.tile as tile
from concourse import masks, mybir
from concourse.bass_utils import run_bass_kernel_spmd

FP = mybir.dt.float32
BF = mybir.dt.bfloat16
AX = mybir.AxisListType
OP = mybir.AluOpType

N = 4096
NCORES = 8
QR = 32            # row chunks of 128 points (pt = 128*j + p)
QC = 4             # col chunks per core (512 cols/core)
W = QR + QC        # merged row+col working width
PEN = float(2**14)
SENT = 60000.0     # min-identity sentinel, bf16-safe


def _emit(tc, out_d2_ap, out_m1_ap, norm_ap, px_ap, py_ap, pz_ap):
    nc = tc.nc

    psf = tc.alloc_tile_pool(name="psf", bufs=6, space="PSUM")
    ptp = tc.alloc_tile_pool(name="ptp", bufs=2, space="PSUM")
    per = tc.alloc_tile_pool(name="per", bufs=1)
    fsp = tc.alloc_tile_pool(name="fsp", bufs=4)
    drm = tc.alloc_tile_pool(name="drm", bufs=1, space="DRAM")

    def _t(shape, name, dt=FP):
        return per.tile(shape, dt, name=name)

    # ---- inputs (norm arrives pre-broadcast to 128 partitions from host —
    # avoids the Pool ucode-library load that partition_broadcast triggers)
    NB = _t([128, 4], "NB")
    nc.scalar.dma_start(NB[:], norm_ap[:])
    PX = _t([128, W], "PX")
    nc.sync.dma_start(PX[:], px_ap[:])
    PY = _t([128, W], "PY")
    nc.sync.dma_start(PY[:], py_ap[:])
    PZ = _t([128, W], "PZ")
    nc.gpsimd.dma_start(PZ[:], pz_ap[:])

    # ---- constants (no input deps; scheduler runs them under the DMAs)
    ident = _t([128, 128], "ident", BF)
    masks.make_identity(nc, ident[:])
    zer_pl = _t([128, QR, 3], "zer_pl", BF)
    nc.gpsimd.memset(zer_pl[:], 0.0)
    CM = _t([128, 512], "CM", BF)
    nc.gpsimd.memset(CM[:], -SENT)
    ACOMP = _t([128, QR, 16], "ACOMP", BF)
    nc.gpsimd.memset(ACOMP[:, :, 14:16], 1.0)
    BCOMP = _t([128, QC, 16], "BCOMP", BF)
    nc.gpsimd.memset(BCOMP[:, :, 12:14], 1.0)

    # ---- plane constants
    nsq = _t([128, 4], "nsq")
    nc.vector.tensor_tensor(nsq[:], NB[:], NB[:], op=OP.mult)
    snn = _t([128, 1], "snn")
    nc.vector.tensor_reduce(snn[:], nsq[:, 0:3], axis=AX.X, op=OP.add)
    inv_nn = _t([128, 1], "inv_nn")
    nc.vector.reciprocal(inv_nn[:], snn[:])
    ninv2 = _t([128, 1], "ninv2")
    nc.scalar.mul(ninv2[:], inv_nn[:], -2.0)
    c4d = _t([128, 1], "c4d")
    nc.vector.tensor_tensor(c4d[:], NB[:, 3:4], inv_nn[:], op=OP.mult)
    nc.scalar.mul(c4d[:], c4d[:], 4.0)
    NCC = _t([128, 3], "NCC")
    nc.vector.tensor_scalar(NCC[:], NB[:, 0:3], ninv2[:], None, op0=OP.mult)

    # ---- plane eval: s = p.n + d over all 36 chunks; m1 = (s<0)
    s_all = _t([128, W], "s_all")
    t1_ = _t([128, W], "t1_")
    nc.scalar.mul(s_all[:], PX[:], NB[:, 0:1])
    nc.scalar.mul(t1_[:], PY[:], NB[:, 1:2])
    nc.vector.tensor_tensor(s_all[:], s_all[:], t1_[:], op=OP.add)
    nc.vector.tensor_scalar(t1_[:], PZ[:], NB[:, 2:3], None, op0=OP.mult)
    nc.vector.tensor_tensor(s_all[:], s_all[:], t1_[:], op=OP.add)
    nc.vector.tensor_scalar_add(s_all[:], s_all[:], NB[:, 3:4])
    M1f = _t([128, W], "M1f")
    nc.vector.tensor_scalar(M1f[:], s_all[:], 0.0, None, op0=OP.is_lt)
    M1 = M1f[:, 0:QR]
    M2CB = _t([128, QC], "M2CB")
    nc.vector.tensor_scalar(
        M2CB[:], M1f[:, QR:W], -1.0, 1.0, op0=OP.mult, op1=OP.add
    )
    nc.scalar.dma_start(out_m1_ap[:], M1)

    # ---- operand vectors: rows a = p + (s*ninv2*n_c), cols b' = -2p
    V3 = _t([128, 3, W], "V3")
    for c, PC in enumerate((PX, PY, PZ)):
        tv = _t([128, QR], f"tv{c}")
        nc.scalar.mul(tv[:], s_all[:, 0:QR], NCC[:, c : c + 1])
        nc.vector.tensor_tensor(
            V3[:, c, 0:QR], tv[:], PC[:, 0:QR], op=OP.add
        )
        nc.vector.tensor_scalar(
            V3[:, c, QR:W], PC[:, QR:W], -2.0, None, op0=OP.mult
        )

    # ---- rr = |p|^2 + (4d/nn)*s (rows) + penalties
    pp = _t([128, W], "pp")
    q1 = _t([128, W], "q1")
    q2 = _t([128, W], "q2")
    nc.vector.tensor_tensor(pp[:], PX[:], PX[:], op=OP.mult)
    nc.gpsimd.tensor_tensor(q1[:], PY[:], PY[:], op=OP.mult)
    nc.gpsimd.tensor_tensor(q2[:], PZ[:], PZ[:], op=OP.mult)
    nc.vector.tensor_tensor(pp[:], pp[:], q1[:], op=OP.add)
    nc.vector.tensor_tensor(pp[:], pp[:], q2[:], op=OP.add)
    t3r = _t([128, QR], "t3r")
    nc.scalar.mul(t3r[:], s_all[:, 0:QR], c4d[:])
    t4r = _t([128, QR], "t4r")
    nc.vector.tensor_scalar(
        t4r[:], M1, -PEN, PEN, op0=OP.mult, op1=OP.add
    )
    t4c = _t([128, QC], "t4c")
    nc.vector.tensor_scalar(t4c[:], M1f[:, QR:W], PEN, None, op0=OP.mult)
    rr = _t([128, W], "rr")
    nc.vector.tensor_tensor(rr[:, 0:QR], pp[:, 0:QR], t3r[:], op=OP.add)
    nc.vector.tensor_tensor(rr[:, 0:QR], rr[:, 0:QR], t4r[:], op=OP.add)
    nc.gpsimd.tensor_tensor(rr[:, QR:W], pp[:, QR:W], t4c[:], op=OP.add)

    # ---- bf16 hi/lo splits (lo = x - hi with direct bf16 operand; lo is
    # placed into the composites with casting copies — no VL3/RRL tiles)
    VH3 = _t([128, 3, W], "VH3", BF)
    nc.scalar.copy(VH3[:], V3[:])
    vlo = _t([128, 3, W], "vlo")
    nc.gpsimd.tensor_tensor(vlo[:], V3[:], VH3[:], op=OP.subtract)
    RRH = _t([128, W], "RRH", BF)
    nc.scalar.copy(RRH[:], rr[:])
    rlo = _t([128, W], "rlo")
    nc.gpsimd.tensor_tensor(rlo[:], rr[:], RRH[:], op=OP.subtract)

    # ---- composite assembly (K slots c-inner, chunk j outer)
    # A slots: [vh vh vl vl rrh rrl 1 1], B slots: [bh bl bh bl 1 1 rrh rrl]
    AHsrc = VH3[:, :, 0:QR].rearrange("p c j -> p j c")
    ALsrc = vlo[:, :, 0:QR].rearrange("p c j -> p j c")
    nc.vector.tensor_copy(ACOMP[:, :, 0:3], AHsrc)
    nc.gpsimd.tensor_tensor(ACOMP[:, :, 3:6], AHsrc, zer_pl[:], op=OP.add)
    nc.scalar.copy(ACOMP[:, :, 6:9], ALsrc)
    nc.vector.tensor_copy(ACOMP[:, :, 9:12], ALsrc)
    nc.vector.tensor_copy(ACOMP[:, :, 12:13], RRH[:, 0:QR])
    nc.scalar.copy(ACOMP[:, :, 13:14], rlo[:, 0:QR])

    BHsrc = VH3[:, :, QR:W].rearrange("p c j -> p j c")
    BLsrc = vlo[:, :, QR:W].rearrange("p c j -> p j c")
    nc.vector.tensor_copy(BCOMP[:, :, 0:3], BHsrc)
    nc.scalar.copy(BCOMP[:, :, 3:6], BLsrc)
    nc.gpsimd.tensor_tensor(
        BCOMP[:, :, 6:9], BHsrc, zer_pl[:, 0:QC, :], op=OP.add
    )
    nc.vector.tensor_copy(BCOMP[:, :, 9:12], BLsrc)
    nc.vector.tensor_copy(BCOMP[:, :, 14:15], RRH[:, QR:W])
    nc.scalar.copy(BCOMP[:, :, 15:16], rlo[:, QR:W])

    # ---- K-major operand images: PE transposes make point index p the
    # contiguous axis, then a compact DRAM bounce regroups partitions
    # (16*dj+c -> c) with 256B-run descriptors (no scatter DMAs).
    # B (the moving operand, gating every matmul) goes first.
    TBT = _t([64, 128], "TBT", BF)
    ptB = ptp.tile([128, 128], BF, tag="tp")
    nc.tensor.transpose(ptB[0:64, :], BCOMP[:], ident[:])
    nc.vector.tensor_copy(TBT[:], ptB[0:64, :])
    stgB = drm.tile([64, 128], BF, name="stgB")
    nc.sync.dma_start(stgB[:], TBT[:])
    TBSB = _t([16, QC, 128], "TBSB", BF)
    nc.sync.dma_start(
        TBSB[:], stgB[:].rearrange("(jc c) p -> c jc p", c=16)
    )
    TASB = _t([16, QR, 128], "TASB", BF)
    # group 0: per-chunk transposes land at base partition 0 directly —
    # the first 8 stationary tiles skip the DRAM bounce entirely.
    for m in range(8):
        ptS = ptp.tile([128, 128], BF, tag="tp")
        nc.tensor.transpose(ptS[0:16, :], ACOMP[:, m, :], ident[:])
        if m % 2 == 0:
            nc.vector.tensor_copy(TASB[:, m, :], ptS[0:16, :])
        else:
            nc.scalar.copy(TASB[:, m, :], ptS[0:16, :])


    # ---- A groups 1-3: transpose + DRAM bounce before the loop
    TAT = _t([128, 3, 128], "TAT", BF)
    stgA = drm.tile([128, 3, 128], BF, name="stgA")
    wq = [nc.gpsimd, nc.scalar, nc.gpsimd]
    rq = [nc.sync, nc.sync, nc.gpsimd]
    for g in range(1, 4):
        ptA = ptp.tile([128, 128], BF, tag="tp")
        nc.tensor.transpose(ptA[:], ACOMP[:, 8 * g : 8 * (g + 1), :], ident[:])
        if g % 2 == 0:
            nc.vector.tensor_copy(TAT[:, g - 1, :], ptA[:])
        else:
            nc.scalar.copy(TAT[:, g - 1, :], ptA[:])
        wq[g - 1].dma_start(stgA[:, g - 1, :], TAT[:, g - 1, :])
        rq[g - 1].dma_start(
            TASB[:, 8 * g : 8 * (g + 1), :],
            stgA[:, g - 1, :].rearrange("(dj c) p -> c dj p", c=16),
        )

    # ---- main loop: one K=16 matmul per (128,512) tile; min reductions
    OUTS = _t([128, QR + 1], "OUTS")
    D2 = OUTS[:, 0:QR]
    for m in range(QR):
        fps = psf.tile([128, 512], FP, tag="mm")
        nc.tensor.matmul(
            fps[:],
            TASB[:, m, :],
            TBSB[:, :, :],
            start=True,
            stop=True,
        )
        FScp = fsp.tile([128, 512], BF, tag="fs")
        nc.scalar.mul(FScp[:], fps[:], -1.0)
        nc.vector.tensor_reduce(
            OUTS[:, m : m + 1], fps[:], axis=AX.X, op=OP.min, negate=True
        )
        nc.vector.tensor_tensor(CM[:], CM[:], FScp[:], op=OP.max)

    # ---- columns: transpose CM chunks; d1 = min over rows via free-axis
    d1R = _t([128, QC], "d1R")
    for jc in range(QC):
        ptC = ptp.tile([128, 128], BF, tag="tp")
        nc.tensor.transpose(
            ptC[:], CM[:, 128 * jc : 128 * (jc + 1)], ident[:]
        )
        nc.vector.tensor_reduce(
            d1R[:, jc : jc + 1], ptC[:], axis=AX.X, op=OP.max
        )
    w1 = _t([128, QC], "w1")
    nc.vector.tensor_tensor(w1[:], d1R[:], M2CB[:], op=OP.mult)
    nc.vector.tensor_reduce(
        OUTS[:, QR : QR + 1], w1[:], axis=AX.X, op=OP.add
    )

    # ---- per-core output [D2 | w1s]; kernel() min-combines on host
    nc.sync.dma_start(out_d2_ap[:, 0:QR], OUTS[:, 0:QR])
    nc.scalar.dma_start(out_d2_ap[:, QR : QR + 1], OUTS[:, QR : QR + 1])

    for p in (psf, ptp, per, fsp, drm):
        p.seal()


_NC = None


def build():
    global _NC
    if _NC is not None:
        return _NC
    nc = bacc.Bacc(
        "TRN2", target_bir_lowering=False, debug=False, num_devices=NCORES
    )
    norm_ap = nc.dram_tensor("norm4", [128, 4], FP, kind="ExternalInput").ap()
    px_ap = nc.dram_tensor("px", [128, W], FP, kind="ExternalInput").ap()
    py_ap = nc.dram_tensor("py", [128, W], FP, kind="ExternalInput").ap()
    pz_ap = nc.dram_tensor("pz", [128, W], FP, kind="ExternalInput").ap()
    out_d2_ap = nc.dram_tensor(
        "out_d2", [128, QR + 1], FP, kind="ExternalOutput"
    ).ap()
    out_m1_ap = nc.dram_tensor("out_m1", [128, QR], FP, kind="ExternalOutput").ap()
    with tile.TileContext(nc) as tc:
        _emit(tc, out_d2_ap, out_m1_ap, norm_ap, px_ap, py_ap, pz_ap)
    nc.compile()
    _NC = nc
    return nc


def make_in_maps(norm, points):
    norm = np.ascontiguousarray(
        np.broadcast_to(np.asarray(norm, dtype=np.float32), (128, 4))
    )
    pts = np.ascontiguousarray(points, dtype=np.float32)
    # rows: pt = 128*j + p  -> PA[p, j, c]
    PTr = pts.reshape(QR, 128, 3).transpose(1, 0, 2)
    maps = []
    for c in range(NCORES):
        cb = (
            pts[512 * c : 512 * (c + 1)]
            .reshape(QC, 128, 3)
            .transpose(1, 0, 2)
        )
        pa = np.concatenate([PTr, cb], axis=1)
        maps.append(
            {
                "norm4": norm,
                "px": np.ascontiguousarray(pa[:, :, 0]),
                "py": np.ascontiguousarray(pa[:, :, 1]),
                "pz": np.ascontiguousarray(pa[:, :, 2]),
            }
        )
    return maps


LAST_RESULTS = None


def kernel(norm, points):
    global LAST_RESULTS
    nc = build()
    maps = make_in_maps(norm, points)
    trace = bool(os.environ.get("KERNEL_TRACE"))
    LAST_RESULTS = run_bass_kernel_spmd(
        nc, maps, list(range(NCORES)), trace=trace
    )
    return combine(LAST_RESULTS.results)


def combine(res):
    outs = np.stack(
        [np.asarray(res[c]["out_d2"], dtype=np.float64) for c in range(NCORES)]
    )
    d2 = outs[:, :, 0:QR]
    w1s = outs[:, :, QR]
    m1 = np.asarray(res[0]["out_m1"], dtype=np.float64)
    min_f = -np.max(d2, axis=0)          # true min_j F[i, j] per row point
    c1 = max(m1.sum(), 1.0)
    c2 = max(float(N) - m1.sum(), 1.0)
    av2_num = float((min_f * m1).sum())
    av1_num = -float(w1s.sum())
    out = 50.0 * (av1_num / c2 + av2_num / c1)
    return np.float32(out)
